# revision 78
# baseline (speedup 1.0000x reference)
"""Trainium2 Bass kernel for a dense transformer block.

Block: h=LN1(x); per-head q/k/v proj; causal softmax attention; x+=sa;
ff=relu(LN2(x)@w_ff+b_ff); out=x+ff.  Shapes: B=4, T=2048, C=512, H=8, d=64.

Sharding (8 cores, one SPMD program): core c -> (batch b=c//2, token
parity par=c%2).  Each core computes full-sequence LN1 + K/V projections
for its batch, and Q/attention/LN2/FFN for its 1024 tokens with
t % 2 == par.  Parity is baked into the *data* (the host rolls the
sequence by -par so own tokens sit at even positions, and sends per-core
causal mask tables), so all 8 cores run the identical program.

Everything on device is transposed ([channels, tokens]); the host
transposes in/out.  LN gamma/beta are folded into projection weights
(exact), the 1/sqrt(d) score scale into wq.  Softmax skips max
subtraction (scores are small here); the denominator comes from a
ones-column appended to V so the attention matmul produces it for free.
Causal masking multiplies exp outputs by a 0/1 mask on GPSIMD (only the
4 diagonal key-blocks of each query strip need it).
"""

import os

import numpy as np

import concourse.bass as bass
import concourse.bacc as bacc
import concourse.mybir as mybir
import concourse.tile as tile
from concourse import bass_isa
from concourse.bass_utils import run_bass_kernel_spmd

dt = mybir.dt
AF = mybir.ActivationFunctionType
ALU = mybir.AluOpType
F32 = dt.float32
F32R = dt.float32r

B, T, C, H, D = 4, 2048, 512, 8, 64
TQ = T // 2            # queries per core
NJ = C // 128          # 4 channel tiles
NTK = T // 128         # 16 key blocks
EPS = 1e-5
STRIP = 256            # local query strip width
NSTRIP = TQ // STRIP   # 4
N_CORES = 8
WRAP = int(os.environ.get("WRAP", "1"))  # 0 disables the wraparound key-0 fix

_CACHE = {}


def _build(taps=False):
    nc = bacc.Bacc("TRN2", target_bir_lowering=False, debug=False,
                   num_devices=N_CORES)

    def inp(name, shape, dtype=F32):
        return nc.dram_tensor(name, shape, dtype, kind="ExternalInput")

    d_in = {
        "xT": inp("xT", [128, NJ * T], F32R),
        "wq": inp("wq_eff", [128, NJ * C], F32R),
        "wk": inp("wk_eff", [128, NJ * C], F32R),
        "wv": inp("wv_eff", [128, NJ * C], F32R),
        "wf": inp("wf_eff", [128, NJ * C], F32R),
        "uq": inp("u_q", [128, NJ]),
        "uk": inp("u_k", [128, NJ]),
        "uvb": inp("u_v_b", [128, C]),
        "wb": inp("w_b", [128, NJ]),
        "mask": inp("mask01", [128, 2 * 8 * STRIP], dt.bfloat16),
        "wrow": inp("wrow", [128, 1]),
        "ones": inp("ones", [128, NTK * H], F32R),
    }
    out_d = nc.dram_tensor("outT", [128, NJ * TQ], F32, kind="ExternalOutput")
    stats_dram = nc.dram_tensor("stats_bounce", [128, 32], F32)
    tap_d = None
    if taps:
        tap_d = {name: nc.dram_tensor("tap_" + name, shape, F32, kind="ExternalOutput")
                 for name, shape in (("rb", [128, T]), ("m2b", [128, T]),
                                     ("hT", [128, NJ * T]), ("qT", [128, NJ * TQ]),
                                     ("kT", [128, NJ * T]), ("va", [128, NTK * H * (D + 1)]),
                                     ("x2T", [128, NSTRIP * NJ * STRIP]),
                                     ("P0", [128, 8 * STRIP]))}

    with tile.TileContext(nc, pool_alloc_mode="queue") as tc:
        _body(nc, tc, d_in, out_d, stats_dram, tap_d)
    nc.compile()
    return nc


def _body(nc, tc, d_in, out_d, stats_dram, tap_d=None):
    from contextlib import ExitStack
    ctx = ExitStack()
    with ctx:
        consts = ctx.enter_context(tc.tile_pool(name="consts", bufs=1))
        wpool = ctx.enter_context(tc.tile_pool(name="w", bufs=1))
        qpool = ctx.enter_context(tc.tile_pool(name="q", bufs=1))
        hpool = ctx.enter_context(tc.tile_pool(name="h", bufs=1))
        pspool = ctx.enter_context(tc.tile_pool(name="ps", bufs=1, space="PSUM"))
        score_ps = ctx.enter_context(tc.tile_pool(name="score_ps", bufs=1, space="PSUM"))
        attn_ps = ctx.enter_context(tc.tile_pool(name="attn_ps", bufs=1, space="PSUM"))
        xpool_cm = tc.tile_pool(name="x", bufs=1)
        xpool = xpool_cm.__enter__()
        ln1pool_cm = tc.tile_pool(name="ln1", bufs=1)
        ln1pool = ln1pool_cm.__enter__()

        # ---- weights / constants ----
        wq = wpool.tile([128, NJ, C], F32R, tag="wA")
        wk = wpool.tile([128, NJ, C], F32R, tag="wB")
        wv = wpool.tile([128, NJ, C], F32R, tag="wC")
        uq = consts.tile([128, NJ], F32, tag="uq")
        uk = consts.tile([128, NJ], F32, tag="uk")
        uvb = consts.tile([128, C], F32, tag="uvb")
        wb = consts.tile([128, NJ], F32, tag="wb")
        mask = consts.tile([128, 2, 8, STRIP], dt.bfloat16, tag="mask")
        wrow = consts.tile([128, 1], F32, tag="wrow")
        nc.sync.dma_start(wrow[:], d_in["wrow"].ap())
        for tdst, key in ((wq, "wq"), (wk, "wk"), (wv, "wv")):
            nc.sync.dma_start(tdst[:].rearrange("p a b -> p (a b)"), d_in[key].ap())
        nc.sync.dma_start(uq[:], d_in["uq"].ap())
        nc.sync.dma_start(uk[:], d_in["uk"].ap())
        nc.sync.dma_start(uvb[:], d_in["uvb"].ap())
        nc.sync.dma_start(wb[:], d_in["wb"].ap())
        nc.sync.dma_start(mask[:].rearrange("p a c b -> p (a c b)"),
                          d_in["mask"].ap())
        eps_t = consts.tile([128, 1], F32, tag="eps")
        nc.gpsimd.memset(eps_t[:], EPS)
        ones1 = consts.tile([128, 2], F32R, tag="ones1")
        nc.sync.dma_start(ones1[:], d_in["ones"].ap()[:, 0:2])

        xT = xpool.tile([128, NJ, T], F32R, tag="xT")
        nc.sync.dma_start(xT[:].rearrange("p a b -> p (a b)"), d_in["xT"].ap())

        # ---- LN1 stats: channel sums / sq-sums via PE ones-matmul on xT ----
        r_b = ln1pool.tile([128, T], F32, tag="r_b")
        m2_b = ln1pool.tile([128, T], F32, tag="m2_b")
        for cch in range(4):
            cols = slice(cch * 512, (cch + 1) * 512)
            xsq = ln1pool.tile([128, NJ, 512], F32R, tag="xsq", bufs=2)
            nc.scalar.activation(xsq[:], xT[:, :, cols], AF.Square)
            sta = pspool.tile([128, 512], F32, tag="proj", bufs=2, name=f"stat_a{cch}")
            stb = pspool.tile([128, 512], F32, tag="proj", bufs=2, name=f"stat_b{cch}")
            for j in range(NJ):
                nc.tensor.matmul(sta[0:2, :], ones1[:], xT[:, j, cols],
                                 start=(j == 0), stop=(j == NJ - 1))
                nc.tensor.matmul(stb[0:2, :], ones1[:], xsq[:, j, :],
                                 start=(j == 0), stop=(j == NJ - 1))
            mu = ln1pool.tile([1, 512], F32, tag="mu", bufs=2)
            nc.vector.tensor_scalar(mu[:], sta[0:1, :], 1.0 / C, None, op0=ALU.mult)
            musq = ln1pool.tile([1, 512], F32, tag="musq", bufs=2)
            nc.vector.tensor_tensor(musq[:], mu[:], mu[:], ALU.mult)
            var = ln1pool.tile([1, 512], F32, tag="var", bufs=2)
            nc.vector.scalar_tensor_tensor(var[:], stb[0:1, :], 1.0 / C, musq[:],
                                           op0=ALU.mult, op1=ALU.subtract)
            sd = ln1pool.tile([1, 512], F32, tag="sd", bufs=2)
            nc.scalar.activation(sd[:], var[:], AF.Sqrt, bias=eps_t[0:1, :])
            nc.vector.reciprocal(r_b[0:1, cols], sd[:])
            nc.vector.tensor_tensor(m2_b[0:1, cols], mu[:], r_b[0:1, cols], ALU.mult)
        nc.gpsimd.partition_broadcast(r_b[:], r_b[0:1, :])
        nc.gpsimd.partition_broadcast(m2_b[:], m2_b[0:1, :])

        if tap_d:
            nc.sync.dma_start(tap_d["rb"].ap(), r_b[:])
            nc.sync.dma_start(tap_d["m2b"].ap(), m2_b[:])
        # ---- LN1 apply: hT = xT*r - mean*r ----
        hT = hpool.tile([128, NJ, T], F32R, tag="hT")
        for j in range(NJ):
            for hf in range(2):
                cols = slice(hf * (T // 2), (hf + 1) * (T // 2))
                tmp = ln1pool.tile([128, T // 2], F32, tag="lntmp")
                nc.vector.tensor_tensor(tmp[:], xT[:, j, cols], r_b[:, cols], ALU.mult)
                nc.vector.tensor_tensor(hT[:, j, cols], tmp[:], m2_b[:, cols], ALU.subtract)

        def h_own(jk):  # even (own-token) columns of hT tile jk
            return hT[:, jk, :].rearrange("p (t two) -> p two t", two=2)[:, 0, :]

        # ---- Q projection (own tokens) ----
        qT = qpool.tile([128, NJ, TQ], F32R, tag="qT")
        for jo in range(NJ):
            for n in range(TQ // 512):
                pq = pspool.tile([128, 512], F32, tag="proj", bufs=2)
                for jk in range(NJ):
                    nc.tensor.matmul(pq[:], wq[:, jk, jo * 128:(jo + 1) * 128],
                                     h_own(jk)[:, n * 512:(n + 1) * 512],
                                     start=(jk == 0), stop=(jk == NJ - 1))
                nc.vector.tensor_scalar(qT[:, jo, n * 512:(n + 1) * 512], pq[:],
                                        uq[:, jo:jo + 1], None, op0=ALU.add)

        if tap_d:
            nc.sync.dma_start(tap_d["hT"].ap(), hT[:].rearrange("p a b -> p (a b)").bitcast(F32))
            nc.sync.dma_start(tap_d["qT"].ap(), qT[:].rearrange("p a b -> p (a b)").bitcast(F32))
        ln1pool_cm.__exit__(None, None, None)
        xpool_cm.__exit__(None, None, None)

        kpool = ctx.enter_context(tc.tile_pool(name="k", bufs=1))
        vapool = ctx.enter_context(tc.tile_pool(name="va", bufs=1))
        ppool = ctx.enter_context(tc.tile_pool(name="p", bufs=1))
        x2pool = ctx.enter_context(tc.tile_pool(name="x2", bufs=1))
        spool = ctx.enter_context(tc.tile_pool(name="small", bufs=1))
        wrappool = ctx.enter_context(tc.tile_pool(name="wrapp", bufs=1))
        vawpool = ctx.enter_context(tc.tile_pool(name="vaw", bufs=1))

        kT = kpool.tile([128, NJ, T], F32R, tag="kT")
        va = vapool.tile([128, NTK, H, D + 1], F32R, tag="va")
        nc.sync.dma_start(va[:, :, :, D:D + 1], d_in["ones"].ap())
        x2T = x2pool.tile([128, NSTRIP, NJ, STRIP], F32R, tag="x2T")

        def kv_chunk(tkc):
            tcol = slice(tkc * 512, (tkc + 1) * 512)
            for jo in range(NJ):
                pk = pspool.tile([128, 512], F32, tag="proj", bufs=2)
                for jk in range(NJ):
                    nc.tensor.matmul(pk[:], wk[:, jk, jo * 128:(jo + 1) * 128],
                                     hT[:, jk, tcol], start=(jk == 0), stop=(jk == NJ - 1))
                nc.vector.tensor_scalar(kT[:, jo, tcol], pk[:],
                                        uk[:, jo:jo + 1], None, op0=ALU.add)
            for tt in range(tkc * 4, tkc * 4 + 4):
                pv = pspool.tile([128, 512], F32, tag="proj", bufs=2)
                for jk in range(NJ):
                    nc.tensor.matmul(pv[:], hT[:, jk, tt * 128:(tt + 1) * 128],
                                     wv[:, jk, :], start=(jk == 0), stop=(jk == NJ - 1))
                nc.vector.tensor_tensor(
                    va[:, tt, :, 0:D],
                    pv[:].rearrange("p (h d) -> p h d", d=D),
                    uvb[:].rearrange("p (h d) -> p h d", d=D), ALU.add)

        def attn_strip(s):
            qcol = slice(s * STRIP, (s + 1) * STRIP)
            nc.sync.dma_start(
                x2T[:, s, :, :],
                d_in["xT"].ap().rearrange("p (a t two) -> p a two t", a=NJ, two=2)[:, :, 0, qcol])
            P2w = None
            if s < 3 and WRAP:
                # wraparound block 15: parity-1 cores hold global key 0 at
                # local key 2047, causal for every query.  The 0/1 row-mask
                # is folded into va_wrap, so no P-side masking is needed.
                kc15 = slice(15 * 128, 16 * 128)
                sc2 = score_ps.tile([128, 8, STRIP], F32, tag="sc",
                                    name=f"sc2_{s}")
                for pr in range(NJ):
                    nc.tensor.matmul(sc2[:, pr, :], kT[0:64, pr, kc15],
                                     qT[0:64, pr, qcol], start=True, stop=True,
                                     tile_position=(0, 0))
                    nc.tensor.matmul(sc2[:, 4 + pr, :], kT[64:128, pr, kc15],
                                     qT[64:128, pr, qcol], start=True, stop=True,
                                     tile_position=(64, 0))
                P2w = wrappool.tile([128, 8, STRIP], F32R, tag="P2w",
                                    name=f"P2w_{s}")
                nc.scalar.activation(P2w[:].rearrange("p a b -> p (a b)"),
                                     sc2[:].rearrange("p a b -> p (a b)"),
                                     AF.Exp)
            for pair in range(NJ):
                h0, h1 = 2 * pair, 2 * pair + 1
                at0t = attn_ps.tile([65, STRIP], F32, tag="attn", bufs=2,
                                    name=f"at0_{s}_{pair}")
                at1t = attn_ps.tile([65, STRIP], F32, tag="attn", bufs=2,
                                    name=f"at1_{s}_{pair}")
                at0, at1 = at0t[:], at1t[:]
                def at_group(g, P):
                    for i in range(4):
                        kb = 4 * g + i
                        first = (g == 0 and i == 0)
                        last = (g == s and i == 3 and (s == 3 or not WRAP))
                        nc.tensor.matmul(at0, va[:, kb, h0, :], P[:, i, :],
                                         start=first, stop=last)
                        nc.tensor.matmul(at1, va[:, kb, h1, :], P[:, 4 + i, :],
                                         start=first, stop=last)

                P_prev = None
                for g in range(s + 1):
                    sc = score_ps.tile([128, 8, STRIP], F32, tag="sc")
                    for i in range(4):
                        kb = 4 * g + i
                        kcol = slice(kb * 128, (kb + 1) * 128)
                        nc.tensor.matmul(sc[:, i, :], kT[0:64, pair, kcol],
                                         qT[0:64, pair, qcol], start=True, stop=True,
                                         tile_position=(0, 0))
                        nc.tensor.matmul(sc[:, 4 + i, :], kT[64:128, pair, kcol],
                                         qT[64:128, pair, qcol], start=True, stop=True,
                                         tile_position=(64, 0))
                    if P_prev is not None:
                        # issue last group's attn matmuls after this group's
                        # scores so the PE is not head-of-line blocked on exp
                        at_group(g - 1, P_prev)
                    P = ppool.tile([128, 8, STRIP], F32R, tag="P", bufs=2)
                    nc.scalar.activation(P[:].rearrange("p a b -> p (a b)"),
                                         sc[:].rearrange("p a b -> p (a b)"), AF.Exp)
                    if tap_d is not None and s == 0 and pair == 0 and g == 0:
                        nc.sync.dma_start(tap_d["P0"].ap(),
                                          P[:].rearrange("p a b -> p (a b)").bitcast(F32))
                    if g == s:  # diagonal group: zero non-causal entries
                        seg = P[:].rearrange("p a b -> p (a b)")
                        nc.gpsimd.tensor_tensor(
                            seg, seg,
                            mask[:, 1 if s == 3 else 0, :, :].rearrange("p a b -> p (a b)"), ALU.mult)
                    P_prev = P
                at_group(s, P_prev)
                if s < 3 and WRAP:
                    nc.tensor.matmul(at0, va_wrap[:, h0, :], P2w[:, pair, :],
                                     start=False, stop=True)
                    nc.tensor.matmul(at1, va_wrap[:, h1, :], P2w[:, 4 + pair, :],
                                     start=False, stop=True)
                for at, hid in ((at0, h0), (at1, h1)):
                    rows = slice(64 * (hid % 2), 64 * (hid % 2) + 64)
                    rd = spool.tile([1, STRIP], F32, tag="rd", bufs=1)
                    nc.vector.reciprocal(rd[:], at[64:65, :])
                    rb = spool.tile([128, STRIP], F32, tag="rb", bufs=1)
                    nc.gpsimd.partition_broadcast(rb[:], rd[:])
                    tmp = spool.tile([128, STRIP], F32, tag="satmp", bufs=1)
                    nc.vector.tensor_tensor(tmp[rows, :], at[0:64, :], rb[rows, :],
                                            ALU.mult)
                    nc.vector.tensor_tensor(x2T[rows, s, hid // 2, :], tmp[rows, :],
                                            x2T[rows, s, hid // 2, :], ALU.add)

        def post_strip(s, wf, ps_):
            qcol = slice(s * STRIP, (s + 1) * STRIP)
            xs = x2T[:, s, :, :]

            def seg(lo, n):  # [128, n] scratch slice
                return ps_[:, lo:lo + n]

            sq2t = ppool.tile([128, 8, STRIP], F32R, tag="P", bufs=2,
                              name=f"sq2_{s}")
            sq2 = sq2t[:, 0:NJ, :]
            nc.vector.tensor_tensor(sq2.rearrange("p a b -> p (a b)"),
                                    xs.rearrange("p a b -> p (a b)"),
                                    xs.rearrange("p a b -> p (a b)"), ALU.mult)
            sta = pspool.tile([128, 512], F32, tag="proj", bufs=2, name=f"psta{s}")
            stb = pspool.tile([128, 512], F32, tag="proj", bufs=2, name=f"pstb{s}")
            for j in range(NJ):
                nc.tensor.matmul(sta[0:2, 0:STRIP], ones1[:], xs[:, j, :],
                                 start=(j == 0), stop=(j == NJ - 1))
                nc.tensor.matmul(stb[0:2, 0:STRIP], ones1[:], sq2[:, j, :],
                                 start=(j == 0), stop=(j == NJ - 1))
            stt = spool.tile([1, 6 * STRIP], F32, tag="pstats", bufs=1)
            mu2, musq, var2, sd2, r2r, m2r = (
                stt[0:1, k * STRIP:(k + 1) * STRIP] for k in range(6))
            nc.vector.tensor_scalar(mu2, sta[0:1, 0:STRIP], 1.0 / C, None,
                                    op0=ALU.mult)
            nc.vector.tensor_tensor(musq, mu2, mu2, ALU.mult)
            nc.vector.scalar_tensor_tensor(var2, stb[0:1, 0:STRIP], 1.0 / C,
                                           musq, op0=ALU.mult, op1=ALU.subtract)
            nc.scalar.activation(sd2, var2, AF.Sqrt, bias=eps_t[0:1, :])
            nc.vector.reciprocal(r2r, sd2)
            nc.vector.tensor_tensor(m2r, mu2, r2r, ALU.mult)
            r2 = spool.tile([128, STRIP], F32, tag="r2", bufs=1)
            nc.gpsimd.partition_broadcast(r2[:], r2r)
            m22 = spool.tile([128, STRIP], F32, tag="m22", bufs=1)
            nc.gpsimd.partition_broadcast(m22[:], m2r)
            h2t = spool.tile([128, NJ, STRIP], F32R, tag="h2", bufs=1)
            h2 = h2t[:]
            for j in range(NJ):
                t_ = spool.tile([128, STRIP], F32, tag="lntmp2", bufs=1)
                nc.vector.tensor_tensor(t_[:], xs[:, j, :], r2[:], ALU.mult)
                nc.vector.tensor_tensor(h2[:, j, :], t_[:], m22[:], ALU.subtract)
            outs = seg(4096, NJ * STRIP).rearrange("p (a b) -> p a b", a=NJ)
            for jo in range(NJ):
                pf = pspool.tile([128, 512], F32, tag="proj", bufs=2)
                for jk in range(NJ):
                    nc.tensor.matmul(pf[:, 0:STRIP], wf[:, jk, jo * 128:(jo + 1) * 128],
                                     h2[:, jk, :], start=(jk == 0), stop=(jk == NJ - 1))
                relu = spool.tile([128, STRIP], F32, tag="relu", bufs=1)
                nc.vector.tensor_scalar(relu[:], pf[:, 0:STRIP], wb[:, jo:jo + 1], 0.0,
                                        op0=ALU.add, op1=ALU.max)
                nc.vector.tensor_tensor(outs[:, jo, :], relu[:], xs[:, jo, :], ALU.add)
            nc.sync.dma_start(
                out_d.ap().rearrange("p (a b) -> p a b", a=NJ)[:, :, qcol],
                outs)

        kv_chunk(0)
        kv_chunk(1)
        kv_chunk(3)
        va_wrap = vawpool.tile([128, H, D + 1], F32R, tag="vaw")
        nc.vector.tensor_scalar(va_wrap[:].rearrange("p a b -> p (a b)"),
                                va[:, 15, :, :].rearrange("p a b -> p (a b)"),
                                wrow[:, 0:1], None, op0=ALU.mult)
        attn_strip(0)
        kv_chunk(2)
        wf = wpool.tile([128, NJ, C], F32R, tag="wA")
        nc.sync.dma_start(wf[:].rearrange("p a b -> p (a b)"), d_in["wf"].ap())
        post_scratch = hpool.tile([128, NJ * T], F32, tag="hT", name="post_scratch")
        post_strip(0, wf, post_scratch[:])
        attn_strip(1)
        if tap_d:
            nc.sync.dma_start(tap_d["kT"].ap(), kT[:].rearrange("p a b -> p (a b)").bitcast(F32))
            nc.sync.dma_start(tap_d["va"].ap(), va[:].rearrange("p a h d -> p (a h d)").bitcast(F32))
        attn_strip(2)
        post_strip(1, wf, post_scratch[:])
        attn_strip(3)
        post_strip(2, wf, post_scratch[:])
        post_strip(3, wf, post_scratch[:])
        if tap_d:
            nc.sync.dma_start(tap_d["x2T"].ap(), x2T[:].rearrange("p a c b -> p (a c b)"))


def _prep_inputs(x, wq, wk, wv, w_ff, b_ff, ln1_g, ln1_b, ln2_g, ln2_b):
    f = np.float32
    wq_all = np.ascontiguousarray(wq.transpose(1, 0, 2).reshape(C, C)).astype(f)
    wk_all = np.ascontiguousarray(wk.transpose(1, 0, 2).reshape(C, C)).astype(f)
    wv_all = np.ascontiguousarray(wv.transpose(1, 0, 2).reshape(C, C)).astype(f)
    scale = f(1.0 / np.sqrt(D))
    wq_eff = (ln1_g[:, None] * wq_all * scale).astype(f)
    u_q = (ln1_b @ wq_all * scale).astype(f)
    wk_eff = (ln1_g[:, None] * wk_all).astype(f)
    u_k = (ln1_b @ wk_all).astype(f)
    wv_eff = (ln1_g[:, None] * wv_all).astype(f)
    u_v = (ln1_b @ wv_all).astype(f)
    wf_eff = (ln2_g[:, None] * w_ff).astype(f)
    w_b = (ln2_b @ w_ff + b_ff).astype(f)

    def ktiles(w):  # [C, M] -> [128, NJ*M] (K-tile fold)
        return np.ascontiguousarray(
            w.reshape(NJ, 128, -1).transpose(1, 0, 2).reshape(128, -1)).astype(f)

    def ptile(v):  # [C] -> [128, NJ]
        return np.ascontiguousarray(v.reshape(NJ, 128).T).astype(f)

    common = {
        "wq_eff": ktiles(wq_eff), "wk_eff": ktiles(wk_eff),
        "wv_eff": ktiles(wv_eff), "wf_eff": ktiles(wf_eff),
        "u_q": ptile(u_q), "u_k": ptile(u_k),
        "u_v_b": np.ascontiguousarray(np.tile(u_v, (128, 1))).astype(f),
        "w_b": ptile(w_b),
    }
    p = np.arange(128)[:, None, None, None]
    ss = np.array([0, 3])[None, :, None, None]   # strip variants: s<3 and s==3
    kr = np.arange(4)[None, None, :, None]
    ff = np.arange(STRIP)[None, None, None, :]
    ik = 128 * (4 * ss + kr) + p
    in_maps = []
    for c in range(N_CORES):
        b, par = c // 2, c % 2
        xb = np.asarray(x[b], dtype=f)
        if par:
            xb = np.roll(xb, -1, axis=0)
        xT = np.ascontiguousarray(
            xb.T.reshape(NJ, 128, T).transpose(1, 0, 2).reshape(128, -1))
        tk_g = (ik + par) % T
        tq_g = 2 * (STRIP * ss + ff) + par
        import ml_dtypes
        m4 = (tk_g <= tq_g).astype(ml_dtypes.bfloat16)  # [128, 2, 4, STRIP]
        m01 = np.ascontiguousarray(
            np.concatenate([m4, m4], axis=2).reshape(128, -1))
        wr = np.zeros((128, 1), dtype=f)
        if par:
            wr[127, 0] = 1.0
        m = dict(common)
        m.update({"xT": xT, "mask01": m01, "wrow": wr,
                  "ones": np.ones((128, NTK * H), dtype=f)})
        in_maps.append(m)
    return in_maps


def kernel(x, wq, wk, wv, w_ff, b_ff, ln1_g, ln1_b, ln2_g, ln2_b):
    if "nc" not in _CACHE:
        _CACHE["nc"] = _build()
    nc = _CACHE["nc"]
    in_maps = _prep_inputs(np.asarray(x), np.asarray(wq), np.asarray(wk),
                           np.asarray(wv), np.asarray(w_ff), np.asarray(b_ff),
                           np.asarray(ln1_g), np.asarray(ln1_b),
                           np.asarray(ln2_g), np.asarray(ln2_b))
    res = run_bass_kernel_spmd(nc, in_maps, list(range(N_CORES)))
    out = np.empty((B, T, C), dtype=np.float32)
    for c in range(N_CORES):
        b, par = c // 2, c % 2
        oT = res.results[c]["outT"].reshape(128, NJ, TQ)
        o = oT.transpose(2, 1, 0).reshape(TQ, C)
        tok = (np.arange(TQ) * 2 + par) % T
        out[b, tok, :] = o
    return out



# revision 79
# speedup vs baseline: 1.1520x; 1.1520x over previous
"""Trainium2 Bass kernel for a dense transformer block.

Block: h=LN1(x); per-head q/k/v proj; causal softmax attention; x+=sa;
ff=relu(LN2(x)@w_ff+b_ff); out=x+ff.  Shapes: B=4, T=2048, C=512, H=8, d=64.

Sharding (8 cores, one SPMD program): core c -> (batch b=c//2, token
parity par=c%2).  Each core computes full-sequence LN1 + K/V projections
for its batch, and Q/attention/LN2/FFN for its 1024 tokens with
t % 2 == par.  Parity is baked into the *data* (the host rolls the
sequence by -par so own tokens sit at even positions, and sends per-core
causal mask tables), so all 8 cores run the identical program.

Everything on device is transposed ([channels, tokens]); the host
transposes in/out.  LN gamma/beta are folded into projection weights
(exact), the 1/sqrt(d) score scale into wq.  Softmax skips max
subtraction (scores are small here); the denominator comes from a
ones-column appended to V so the attention matmul produces it for free.
Causal masking multiplies exp outputs by a 0/1 mask on GPSIMD (only the
4 diagonal key-blocks of each query strip need it).
"""

import os

import numpy as np

import concourse.bass as bass
import concourse.bacc as bacc
import concourse.mybir as mybir
import concourse.tile as tile
from concourse import bass_isa
from concourse.bass_utils import run_bass_kernel_spmd

dt = mybir.dt
AF = mybir.ActivationFunctionType
ALU = mybir.AluOpType
F32 = dt.float32
F32R = dt.float32r

B, T, C, H, D = 4, 2048, 512, 8, 64
TQ = T // 2            # queries per core
NJ = C // 128          # 4 channel tiles
NTK = T // 128         # 16 key blocks
EPS = 1e-5
STRIP = 256            # local query strip width
NSTRIP = TQ // STRIP   # 4
N_CORES = 8
WRAP = int(os.environ.get("WRAP", "1"))  # 0 disables the wraparound key-0 fix

_CACHE = {}


def _build(taps=False):
    nc = bacc.Bacc("TRN2", target_bir_lowering=False, debug=False,
                   num_devices=N_CORES)

    def inp(name, shape, dtype=F32):
        return nc.dram_tensor(name, shape, dtype, kind="ExternalInput")

    d_in = {
        "xT": inp("xT", [128, NJ * T], F32R),
        "wq": inp("wq_eff", [128, NJ * C], F32R),
        "wk": inp("wk_eff", [128, NJ * C], F32R),
        "wv": inp("wv_eff", [128, NJ * C], F32R),
        "wf": inp("wf_eff", [128, NJ * C], F32R),
        "uq": inp("u_q", [128, NJ]),
        "uk": inp("u_k", [128, NJ]),
        "uvb": inp("u_v_b", [128, C]),
        "wb": inp("w_b", [128, NJ]),
        "mask": inp("mask01", [128, 2 * 8 * STRIP], dt.bfloat16),
        "wrow": inp("wrow", [128, 1]),
        "ones": inp("ones", [128, NTK * H], F32R),
    }
    out_d = nc.dram_tensor("outT", [128, NJ * TQ], F32, kind="ExternalOutput")
    stats_dram = nc.dram_tensor("stats_bounce", [128, 32], F32)
    tap_d = None
    if taps:
        tap_d = {name: nc.dram_tensor("tap_" + name, shape, F32, kind="ExternalOutput")
                 for name, shape in (("rb", [128, T]), ("m2b", [128, T]),
                                     ("hT", [128, NJ * T]), ("qT", [128, NJ * TQ]),
                                     ("kT", [128, NJ * T]), ("va", [128, NTK * H * (D + 1)]),
                                     ("x2T", [128, NSTRIP * NJ * STRIP]),
                                     ("P0", [128, 8 * STRIP]))}

    with tile.TileContext(nc, pool_alloc_mode="queue") as tc:
        _body(nc, tc, d_in, out_d, stats_dram, tap_d)
    nc.compile()
    return nc


def _body(nc, tc, d_in, out_d, stats_dram, tap_d=None):
    from contextlib import ExitStack
    ctx = ExitStack()
    with ctx:
        consts = ctx.enter_context(tc.tile_pool(name="consts", bufs=1))
        wpool = ctx.enter_context(tc.tile_pool(name="w", bufs=1))
        qpool = ctx.enter_context(tc.tile_pool(name="q", bufs=1))
        hpool = ctx.enter_context(tc.tile_pool(name="h", bufs=1))
        pspool = ctx.enter_context(tc.tile_pool(name="ps", bufs=1, space="PSUM"))
        score_ps = ctx.enter_context(tc.tile_pool(name="score_ps", bufs=1, space="PSUM"))
        attn_ps = ctx.enter_context(tc.tile_pool(name="attn_ps", bufs=1, space="PSUM"))
        xpool_cm = tc.tile_pool(name="x", bufs=1)
        xpool = xpool_cm.__enter__()
        ln1pool_cm = tc.tile_pool(name="ln1", bufs=1)
        ln1pool = ln1pool_cm.__enter__()

        # ---- weights / constants ----
        wq = wpool.tile([128, NJ, C], F32R, tag="wA")
        wk = wpool.tile([128, NJ, C], F32R, tag="wB")
        wv = wpool.tile([128, NJ, C], F32R, tag="wC")
        uq = consts.tile([128, NJ], F32, tag="uq")
        uk = consts.tile([128, NJ], F32, tag="uk")
        uvb = consts.tile([128, C], F32, tag="uvb")
        wb = consts.tile([128, NJ], F32, tag="wb")
        mask = consts.tile([128, 2, 8, STRIP], dt.bfloat16, tag="mask")
        wrow = consts.tile([128, 1], F32, tag="wrow")
        nc.sync.dma_start(wrow[:], d_in["wrow"].ap())
        for tdst, key in ((wq, "wq"), (wk, "wk"), (wv, "wv")):
            nc.sync.dma_start(tdst[:].rearrange("p a b -> p (a b)"), d_in[key].ap())
        nc.sync.dma_start(uq[:], d_in["uq"].ap())
        nc.sync.dma_start(uk[:], d_in["uk"].ap())
        nc.sync.dma_start(uvb[:], d_in["uvb"].ap())
        nc.sync.dma_start(wb[:], d_in["wb"].ap())
        nc.sync.dma_start(mask[:].rearrange("p a c b -> p (a c b)"),
                          d_in["mask"].ap())
        eps_t = consts.tile([128, 1], F32, tag="eps")
        nc.gpsimd.memset(eps_t[:], EPS)
        ones1 = consts.tile([128, 2], F32R, tag="ones1")
        nc.sync.dma_start(ones1[:], d_in["ones"].ap()[:, 0:2])

        xT = xpool.tile([128, NJ, T], F32R, tag="xT")
        nc.sync.dma_start(xT[:].rearrange("p a b -> p (a b)"), d_in["xT"].ap())

        # ---- LN1 stats: channel sums / sq-sums via PE ones-matmul on xT ----
        r_b = ln1pool.tile([128, T], F32, tag="r_b")
        m2_b = ln1pool.tile([128, T], F32, tag="m2_b")
        for cch in range(4):
            cols = slice(cch * 512, (cch + 1) * 512)
            xsq = ln1pool.tile([128, NJ, 512], F32R, tag="xsq", bufs=2)
            nc.scalar.activation(xsq[:], xT[:, :, cols], AF.Square)
            sta = pspool.tile([128, 512], F32, tag="proj", bufs=2, name=f"stat_a{cch}")
            stb = pspool.tile([128, 512], F32, tag="proj", bufs=2, name=f"stat_b{cch}")
            for j in range(NJ):
                nc.tensor.matmul(sta[0:2, :], ones1[:], xT[:, j, cols],
                                 start=(j == 0), stop=(j == NJ - 1))
                nc.tensor.matmul(stb[0:2, :], ones1[:], xsq[:, j, :],
                                 start=(j == 0), stop=(j == NJ - 1))
            mu = ln1pool.tile([1, 512], F32, tag="mu", bufs=2)
            nc.vector.tensor_scalar(mu[:], sta[0:1, :], 1.0 / C, None, op0=ALU.mult)
            musq = ln1pool.tile([1, 512], F32, tag="musq", bufs=2)
            nc.vector.tensor_tensor(musq[:], mu[:], mu[:], ALU.mult)
            var = ln1pool.tile([1, 512], F32, tag="var", bufs=2)
            nc.vector.scalar_tensor_tensor(var[:], stb[0:1, :], 1.0 / C, musq[:],
                                           op0=ALU.mult, op1=ALU.subtract)
            sd = ln1pool.tile([1, 512], F32, tag="sd", bufs=2)
            nc.scalar.activation(sd[:], var[:], AF.Sqrt, bias=eps_t[0:1, :])
            nc.vector.reciprocal(r_b[0:1, cols], sd[:])
            nc.vector.tensor_tensor(m2_b[0:1, cols], mu[:], r_b[0:1, cols], ALU.mult)
        nc.gpsimd.partition_broadcast(r_b[:], r_b[0:1, :])
        nc.gpsimd.partition_broadcast(m2_b[:], m2_b[0:1, :])

        if tap_d:
            nc.sync.dma_start(tap_d["rb"].ap(), r_b[:])
            nc.sync.dma_start(tap_d["m2b"].ap(), m2_b[:])
        # ---- LN1 apply: hT = xT*r - mean*r ----
        hT = hpool.tile([128, NJ, T], F32R, tag="hT")
        for j in range(NJ):
            for hf in range(2):
                cols = slice(hf * (T // 2), (hf + 1) * (T // 2))
                tmp = ln1pool.tile([128, T // 2], F32, tag="lntmp")
                nc.vector.tensor_tensor(tmp[:], xT[:, j, cols], r_b[:, cols], ALU.mult)
                nc.vector.tensor_tensor(hT[:, j, cols], tmp[:], m2_b[:, cols], ALU.subtract)

        def h_own(jk):  # even (own-token) columns of hT tile jk
            return hT[:, jk, :].rearrange("p (t two) -> p two t", two=2)[:, 0, :]

        # ---- Q projection (own tokens) ----
        qT = qpool.tile([128, NJ, TQ], F32R, tag="qT")
        for jo in range(NJ):
            for n in range(TQ // 512):
                pq = pspool.tile([128, 512], F32, tag="proj", bufs=2)
                for jk in range(NJ):
                    nc.tensor.matmul(pq[:], wq[:, jk, jo * 128:(jo + 1) * 128],
                                     h_own(jk)[:, n * 512:(n + 1) * 512],
                                     start=(jk == 0), stop=(jk == NJ - 1))
                nc.vector.tensor_scalar(qT[:, jo, n * 512:(n + 1) * 512], pq[:],
                                        uq[:, jo:jo + 1], None, op0=ALU.add)

        if tap_d:
            nc.sync.dma_start(tap_d["hT"].ap(), hT[:].rearrange("p a b -> p (a b)").bitcast(F32))
            nc.sync.dma_start(tap_d["qT"].ap(), qT[:].rearrange("p a b -> p (a b)").bitcast(F32))
        ln1pool_cm.__exit__(None, None, None)
        xpool_cm.__exit__(None, None, None)

        kpool = ctx.enter_context(tc.tile_pool(name="k", bufs=1))
        vapool = ctx.enter_context(tc.tile_pool(name="va", bufs=1))
        ppool = ctx.enter_context(tc.tile_pool(name="p", bufs=1))
        x2pool = ctx.enter_context(tc.tile_pool(name="x2", bufs=1))
        spool = ctx.enter_context(tc.tile_pool(name="small", bufs=1))
        wrappool = ctx.enter_context(tc.tile_pool(name="wrapp", bufs=1))
        vawpool = ctx.enter_context(tc.tile_pool(name="vaw", bufs=1))

        kT = kpool.tile([128, NJ, T], F32R, tag="kT")
        va = vapool.tile([128, NTK, H, D + 1], F32R, tag="va")
        nc.sync.dma_start(va[:, :, :, D:D + 1], d_in["ones"].ap())
        x2T = x2pool.tile([128, NSTRIP, NJ, STRIP], F32R, tag="x2T")

        def kv_chunk(tkc):
            tcol = slice(tkc * 512, (tkc + 1) * 512)
            for jo in range(NJ):
                pk = pspool.tile([128, 512], F32, tag="proj", bufs=2)
                for jk in range(NJ):
                    nc.tensor.matmul(pk[:], wk[:, jk, jo * 128:(jo + 1) * 128],
                                     hT[:, jk, tcol], start=(jk == 0), stop=(jk == NJ - 1))
                nc.vector.tensor_scalar(kT[:, jo, tcol], pk[:],
                                        uk[:, jo:jo + 1], None, op0=ALU.add)
            for tt in range(tkc * 4, tkc * 4 + 4):
                pv = pspool.tile([128, 512], F32, tag="proj", bufs=2)
                for jk in range(NJ):
                    nc.tensor.matmul(pv[:], hT[:, jk, tt * 128:(tt + 1) * 128],
                                     wv[:, jk, :], start=(jk == 0), stop=(jk == NJ - 1))
                nc.vector.tensor_tensor(
                    va[:, tt, :, 0:D],
                    pv[:].rearrange("p (h d) -> p h d", d=D),
                    uvb[:].rearrange("p (h d) -> p h d", d=D), ALU.add)

        def attn_strip(s):
            qcol = slice(s * STRIP, (s + 1) * STRIP)
            nc.sync.dma_start(
                x2T[:, s, :, :],
                d_in["xT"].ap().rearrange("p (a t two) -> p a two t", a=NJ, two=2)[:, :, 0, qcol])
            P2w = None
            if s < 3 and WRAP:
                # wraparound block 15: parity-1 cores hold global key 0 at
                # local key 2047, causal for every query.  The 0/1 row-mask
                # is folded into va_wrap, so no P-side masking is needed.
                kc15 = slice(15 * 128, 16 * 128)
                sc2 = score_ps.tile([128, 8, STRIP], F32, tag="sc",
                                    name=f"sc2_{s}")
                for pr in range(NJ):
                    nc.tensor.matmul(sc2[:, pr, :], kT[0:64, pr, kc15],
                                     qT[0:64, pr, qcol], start=True, stop=True,
                                     tile_position=(0, 0))
                    nc.tensor.matmul(sc2[:, 4 + pr, :], kT[64:128, pr, kc15],
                                     qT[64:128, pr, qcol], start=True, stop=True,
                                     tile_position=(64, 0))
                P2w = wrappool.tile([128, 8, STRIP], F32R, tag="P2w",
                                    name=f"P2w_{s}")
                nc.scalar.activation(P2w[:].rearrange("p a b -> p (a b)"),
                                     sc2[:].rearrange("p a b -> p (a b)"),
                                     AF.Exp)
            for pair in range(NJ):
                h0, h1 = 2 * pair, 2 * pair + 1
                at0t = attn_ps.tile([65, STRIP], F32, tag="attn", bufs=2,
                                    name=f"at0_{s}_{pair}")
                at1t = attn_ps.tile([65, STRIP], F32, tag="attn", bufs=2,
                                    name=f"at1_{s}_{pair}")
                at0, at1 = at0t[:], at1t[:]
                for g in range(s + 1):
                    sc = score_ps.tile([128, 8, STRIP], F32, tag="sc")
                    for i in range(4):
                        kb = 4 * g + i
                        kcol = slice(kb * 128, (kb + 1) * 128)
                        nc.tensor.matmul(sc[:, i, :], kT[0:64, pair, kcol],
                                         qT[0:64, pair, qcol], start=True, stop=True,
                                         tile_position=(0, 0))
                        nc.tensor.matmul(sc[:, 4 + i, :], kT[64:128, pair, kcol],
                                         qT[64:128, pair, qcol], start=True, stop=True,
                                         tile_position=(64, 0))
                    P = ppool.tile([128, 8, STRIP], F32R, tag="P", bufs=2)
                    nc.scalar.activation(P[:].rearrange("p a b -> p (a b)"),
                                         sc[:].rearrange("p a b -> p (a b)"), AF.Exp)
                    if tap_d is not None and s == 0 and pair == 0 and g == 0:
                        nc.sync.dma_start(tap_d["P0"].ap(),
                                          P[:].rearrange("p a b -> p (a b)").bitcast(F32))
                    if g == s:  # diagonal group: zero non-causal entries
                        seg = P[:].rearrange("p a b -> p (a b)")
                        nc.gpsimd.tensor_tensor(
                            seg, seg,
                            mask[:, 1 if s == 3 else 0, :, :].rearrange("p a b -> p (a b)"), ALU.mult)
                    for i in range(4):
                        kb = 4 * g + i
                        first = (g == 0 and i == 0)
                        last = (g == s and i == 3 and (s == 3 or not WRAP))
                        nc.tensor.matmul(at0, va[:, kb, h0, :], P[:, i, :],
                                         start=first, stop=last)
                        nc.tensor.matmul(at1, va[:, kb, h1, :], P[:, 4 + i, :],
                                         start=first, stop=last)
                if s < 3 and WRAP:
                    nc.tensor.matmul(at0, va_wrap[:, h0, :], P2w[:, pair, :],
                                     start=False, stop=True)
                    nc.tensor.matmul(at1, va_wrap[:, h1, :], P2w[:, 4 + pair, :],
                                     start=False, stop=True)
                for at, hid in ((at0, h0), (at1, h1)):
                    rows = slice(64 * (hid % 2), 64 * (hid % 2) + 64)
                    rd = spool.tile([1, STRIP], F32, tag="rd", bufs=1)
                    nc.vector.reciprocal(rd[:], at[64:65, :])
                    rb = spool.tile([128, STRIP], F32, tag="rb", bufs=1)
                    nc.gpsimd.partition_broadcast(rb[:], rd[:])
                    tmp = spool.tile([128, STRIP], F32, tag="satmp", bufs=1)
                    nc.vector.tensor_tensor(tmp[rows, :], at[0:64, :], rb[rows, :],
                                            ALU.mult)
                    nc.vector.tensor_tensor(x2T[rows, s, hid // 2, :], tmp[rows, :],
                                            x2T[rows, s, hid // 2, :], ALU.add)

        def post_strip(s, wf, ps_):
            qcol = slice(s * STRIP, (s + 1) * STRIP)
            xs = x2T[:, s, :, :]

            def seg(lo, n):  # [128, n] scratch slice
                return ps_[:, lo:lo + n]

            sq2t = ppool.tile([128, 8, STRIP], F32R, tag="P", bufs=2,
                              name=f"sq2_{s}")
            sq2 = sq2t[:, 0:NJ, :]
            nc.vector.tensor_tensor(sq2.rearrange("p a b -> p (a b)"),
                                    xs.rearrange("p a b -> p (a b)"),
                                    xs.rearrange("p a b -> p (a b)"), ALU.mult)
            sta = pspool.tile([128, 512], F32, tag="proj", bufs=2, name=f"psta{s}")
            stb = pspool.tile([128, 512], F32, tag="proj", bufs=2, name=f"pstb{s}")
            for j in range(NJ):
                nc.tensor.matmul(sta[0:2, 0:STRIP], ones1[:], xs[:, j, :],
                                 start=(j == 0), stop=(j == NJ - 1))
                nc.tensor.matmul(stb[0:2, 0:STRIP], ones1[:], sq2[:, j, :],
                                 start=(j == 0), stop=(j == NJ - 1))
            stt = spool.tile([1, 6 * STRIP], F32, tag="pstats", bufs=1)
            mu2, musq, var2, sd2, r2r, m2r = (
                stt[0:1, k * STRIP:(k + 1) * STRIP] for k in range(6))
            nc.vector.tensor_scalar(mu2, sta[0:1, 0:STRIP], 1.0 / C, None,
                                    op0=ALU.mult)
            nc.vector.tensor_tensor(musq, mu2, mu2, ALU.mult)
            nc.vector.scalar_tensor_tensor(var2, stb[0:1, 0:STRIP], 1.0 / C,
                                           musq, op0=ALU.mult, op1=ALU.subtract)
            nc.scalar.activation(sd2, var2, AF.Sqrt, bias=eps_t[0:1, :])
            nc.vector.reciprocal(r2r, sd2)
            nc.vector.tensor_tensor(m2r, mu2, r2r, ALU.mult)
            r2 = spool.tile([128, STRIP], F32, tag="r2", bufs=1)
            nc.gpsimd.partition_broadcast(r2[:], r2r)
            m22 = spool.tile([128, STRIP], F32, tag="m22", bufs=1)
            nc.gpsimd.partition_broadcast(m22[:], m2r)
            h2t = spool.tile([128, NJ, STRIP], F32R, tag="h2", bufs=1)
            h2 = h2t[:]
            for j in range(NJ):
                t_ = spool.tile([128, STRIP], F32, tag="lntmp2", bufs=1)
                nc.vector.tensor_tensor(t_[:], xs[:, j, :], r2[:], ALU.mult)
                nc.vector.tensor_tensor(h2[:, j, :], t_[:], m22[:], ALU.subtract)
            outs = seg(4096, NJ * STRIP).rearrange("p (a b) -> p a b", a=NJ)
            for jo in range(NJ):
                pf = pspool.tile([128, 512], F32, tag="proj", bufs=2)
                for jk in range(NJ):
                    nc.tensor.matmul(pf[:, 0:STRIP], wf[:, jk, jo * 128:(jo + 1) * 128],
                                     h2[:, jk, :], start=(jk == 0), stop=(jk == NJ - 1))
                relu = spool.tile([128, STRIP], F32, tag="relu", bufs=1)
                nc.vector.tensor_scalar(relu[:], pf[:, 0:STRIP], wb[:, jo:jo + 1], 0.0,
                                        op0=ALU.add, op1=ALU.max)
                nc.vector.tensor_tensor(outs[:, jo, :], relu[:], xs[:, jo, :], ALU.add)
            nc.sync.dma_start(
                out_d.ap().rearrange("p (a b) -> p a b", a=NJ)[:, :, qcol],
                outs)

        kv_chunk(0)
        kv_chunk(1)
        kv_chunk(3)
        va_wrap = vawpool.tile([128, H, D + 1], F32R, tag="vaw")
        nc.vector.tensor_scalar(va_wrap[:].rearrange("p a b -> p (a b)"),
                                va[:, 15, :, :].rearrange("p a b -> p (a b)"),
                                wrow[:, 0:1], None, op0=ALU.mult)
        attn_strip(0)
        attn_strip(1)
        kv_chunk(2)
        wf = wpool.tile([128, NJ, C], F32R, tag="wA")
        nc.sync.dma_start(wf[:].rearrange("p a b -> p (a b)"), d_in["wf"].ap())
        post_scratch = hpool.tile([128, NJ * T], F32, tag="hT", name="post_scratch")
        post_strip(0, wf, post_scratch[:])
        if tap_d:
            nc.sync.dma_start(tap_d["kT"].ap(), kT[:].rearrange("p a b -> p (a b)").bitcast(F32))
            nc.sync.dma_start(tap_d["va"].ap(), va[:].rearrange("p a h d -> p (a h d)").bitcast(F32))
        attn_strip(2)
        post_strip(1, wf, post_scratch[:])
        attn_strip(3)
        post_strip(2, wf, post_scratch[:])
        post_strip(3, wf, post_scratch[:])
        if tap_d:
            nc.sync.dma_start(tap_d["x2T"].ap(), x2T[:].rearrange("p a c b -> p (a c b)"))


def _prep_inputs(x, wq, wk, wv, w_ff, b_ff, ln1_g, ln1_b, ln2_g, ln2_b):
    f = np.float32
    wq_all = np.ascontiguousarray(wq.transpose(1, 0, 2).reshape(C, C)).astype(f)
    wk_all = np.ascontiguousarray(wk.transpose(1, 0, 2).reshape(C, C)).astype(f)
    wv_all = np.ascontiguousarray(wv.transpose(1, 0, 2).reshape(C, C)).astype(f)
    scale = f(1.0 / np.sqrt(D))
    wq_eff = (ln1_g[:, None] * wq_all * scale).astype(f)
    u_q = (ln1_b @ wq_all * scale).astype(f)
    wk_eff = (ln1_g[:, None] * wk_all).astype(f)
    u_k = (ln1_b @ wk_all).astype(f)
    wv_eff = (ln1_g[:, None] * wv_all).astype(f)
    u_v = (ln1_b @ wv_all).astype(f)
    wf_eff = (ln2_g[:, None] * w_ff).astype(f)
    w_b = (ln2_b @ w_ff + b_ff).astype(f)

    def ktiles(w):  # [C, M] -> [128, NJ*M] (K-tile fold)
        return np.ascontiguousarray(
            w.reshape(NJ, 128, -1).transpose(1, 0, 2).reshape(128, -1)).astype(f)

    def ptile(v):  # [C] -> [128, NJ]
        return np.ascontiguousarray(v.reshape(NJ, 128).T).astype(f)

    common = {
        "wq_eff": ktiles(wq_eff), "wk_eff": ktiles(wk_eff),
        "wv_eff": ktiles(wv_eff), "wf_eff": ktiles(wf_eff),
        "u_q": ptile(u_q), "u_k": ptile(u_k),
        "u_v_b": np.ascontiguousarray(np.tile(u_v, (128, 1))).astype(f),
        "w_b": ptile(w_b),
    }
    p = np.arange(128)[:, None, None, None]
    ss = np.array([0, 3])[None, :, None, None]   # strip variants: s<3 and s==3
    kr = np.arange(4)[None, None, :, None]
    ff = np.arange(STRIP)[None, None, None, :]
    ik = 128 * (4 * ss + kr) + p
    in_maps = []
    for c in range(N_CORES):
        b, par = c // 2, c % 2
        xb = np.asarray(x[b], dtype=f)
        if par:
            xb = np.roll(xb, -1, axis=0)
        xT = np.ascontiguousarray(
            xb.T.reshape(NJ, 128, T).transpose(1, 0, 2).reshape(128, -1))
        tk_g = (ik + par) % T
        tq_g = 2 * (STRIP * ss + ff) + par
        import ml_dtypes
        m4 = (tk_g <= tq_g).astype(ml_dtypes.bfloat16)  # [128, 2, 4, STRIP]
        m01 = np.ascontiguousarray(
            np.concatenate([m4, m4], axis=2).reshape(128, -1))
        wr = np.zeros((128, 1), dtype=f)
        if par:
            wr[127, 0] = 1.0
        m = dict(common)
        m.update({"xT": xT, "mask01": m01, "wrow": wr,
                  "ones": np.ones((128, NTK * H), dtype=f)})
        in_maps.append(m)
    return in_maps


def kernel(x, wq, wk, wv, w_ff, b_ff, ln1_g, ln1_b, ln2_g, ln2_b):
    if "nc" not in _CACHE:
        _CACHE["nc"] = _build()
    nc = _CACHE["nc"]
    in_maps = _prep_inputs(np.asarray(x), np.asarray(wq), np.asarray(wk),
                           np.asarray(wv), np.asarray(w_ff), np.asarray(b_ff),
                           np.asarray(ln1_g), np.asarray(ln1_b),
                           np.asarray(ln2_g), np.asarray(ln2_b))
    res = run_bass_kernel_spmd(nc, in_maps, list(range(N_CORES)))
    out = np.empty((B, T, C), dtype=np.float32)
    for c in range(N_CORES):
        b, par = c // 2, c % 2
        oT = res.results[c]["outT"].reshape(128, NJ, TQ)
        o = oT.transpose(2, 1, 0).reshape(TQ, C)
        tok = (np.arange(TQ) * 2 + par) % T
        out[b, tok, :] = o
    return out

